# revision 7
# baseline (speedup 1.0000x reference)
"""AddressingHead Trainium2 kernel — 8-core data-parallel (batch sharded).

Per core (B_loc=32, N=8192, Dk=128, Din=512):
  args  = input @ W.T + b                       (PE, f32r)
  query = args[:, :128];  g = sigmoid(args[:,128]);  sd = softmax(args[:,129:134])
  scores[b, n] = memory[b] @ query[b] / sqrt(Dk)
  t1 = exp(scores)  (softmax numerator; Z via fused accum)
  out = conv5(sd, g/Z * t1 + (1-g) * prev)      (circular 5-tap)

Memory tiles [128n x 128d] stream HBM->SBUF (interleaved n = 4p+t layout for
contiguous DMA), are PE-transposed (f32r, into one psum bank per 512-chunk),
copied psum->SBUF (DVE/ACT split), then a masked-stationary f32r matmul
accumulates each batch's score row into a [32, 512] psum tile.
"""
import sys
sys.path.insert(0, "/opt/trn_rl_repo")

import math
from contextlib import ExitStack

import numpy as np

import concourse.bass as bass
import concourse.tile as tile
from concourse import mybir
from concourse.bass_utils import run_bass_kernel_spmd

F32 = mybir.dt.float32
F32R = mybir.dt.float32r
AF = mybir.ActivationFunctionType
OP = mybir.AluOpType

NCORES = 8
B = 256
BLOC = B // NCORES          # 32
DIN = 512
DK = 128
NB = 8192                   # banks (N)
S = 5
ODIM = DK + 1 + S           # 134
NCHUNK = 512                # scores chunk (psum bank width in f32)
SCALE = 1.0 / math.sqrt(DK)

_NO_SPLIT = {"InstNoOp", "InstUnconditionalBranch", "InstCompareAndBranch",
             "InstEventSemOp"}


def split_waits(nc):
    """This toolchain's ISA structs carry only ONE embedded sync-wait slot.
    Hoist extra waits onto standalone same-engine NoOps placed just before."""
    n_split = 0
    for f in nc.m.functions:
        for blk in f.blocks:
            newlist = []
            changed = False
            for inst in blk.instructions:
                if (type(inst).__name__ not in _NO_SPLIT
                        and inst.sync_info is not None
                        and len(inst.sync_info.on_wait) > 1):
                    waits = list(inst.sync_info.on_wait)
                    for j, w in enumerate(waits[:-1]):
                        newlist.append(mybir.InstNoOp(
                            name=f"{inst.name}-ws{j}", engine=inst.engine,
                            sync_info=mybir.SyncInfo(on_wait=[w], on_update=[]),
                            bass_nofuse=True))
                        n_split += 1
                    inst.sync_info = mybir.SyncInfo(
                        on_wait=[waits[-1]],
                        on_update=list(inst.sync_info.on_update))
                    changed = True
                newlist.append(inst)
            if changed:
                blk.instructions = newlist
    return n_split


def build_program(nb=NB, act_memt_copies=True, split=True):
    nchunks = nb // NCHUNK
    nc = bass.Bass("TRN2", target_bir_lowering=False, debug=False)

    inp_d = nc.declare_dram_parameter("input", [BLOC, DIN], F32R, isOutput=False)
    mem_d = nc.declare_dram_parameter("memory", [BLOC, nb, DK], F32R, isOutput=False)
    prev_d = nc.declare_dram_parameter("prev", [BLOC, nb], F32, isOutput=False)
    w0_d = nc.declare_dram_parameter("w0", [DK, DIN], F32R, isOutput=False)
    w1_d = nc.declare_dram_parameter("w1", [ODIM - DK, DIN], F32R, isOutput=False)
    bias_d = nc.declare_dram_parameter("bias", [1, ODIM], F32R, isOutput=False)
    ones_d = nc.declare_dram_parameter("ones", [1, BLOC], F32R, isOutput=False)
    ident_d = nc.declare_dram_parameter("ident", [DK, DK], F32R, isOutput=False)
    out_d = nc.declare_dram_parameter("out", [BLOC, nb], F32, isOutput=True)

    with tile.TileContext(nc) as tc, ExitStack() as ctx:
        cst = ctx.enter_context(tc.tile_pool(name="cst", bufs=1))
        nat_pool = ctx.enter_context(tc.tile_pool(name="nat", bufs=8))
        mt_pool = ctx.enter_context(tc.tile_pool(name="memT", bufs=4))
        ps_a = ctx.enter_context(tc.tile_pool(name="psA", bufs=2, space="PSUM"))
        ps_tp = ctx.enter_context(tc.tile_pool(name="psTP", bufs=3, space="PSUM"))
        ps_sc = ctx.enter_context(tc.tile_pool(name="psSC", bufs=3, space="PSUM"))

        # ---------------- constants / big buffers ----------------
        ident = cst.tile([DK, DK], F32R, tag="ident")
        nc.sync.dma_start(out=ident[:], in_=ident_d[:])
        inp_sb = cst.tile([BLOC, DIN], F32R, tag="inp")
        nc.sync.dma_start(out=inp_sb[:], in_=inp_d[:])
        w0_sb = cst.tile([DK, DIN], F32R, tag="w0")
        nc.sync.dma_start(out=w0_sb[:], in_=w0_d[:])
        w1_sb = cst.tile([ODIM - DK, DIN], F32R, tag="w1")
        nc.sync.dma_start(out=w1_sb[:], in_=w1_d[:])
        bias_sb = cst.tile([1, ODIM], F32R, tag="bias")
        nc.sync.dma_start(out=bias_sb[:], in_=bias_d[:])
        ones_sb = cst.tile([1, BLOC], F32R, tag="ones")
        nc.sync.dma_start(out=ones_sb[:], in_=ones_d[:])

        prevh = cst.tile([BLOC, nb + 4], F32, tag="prevh")
        nc.sync.dma_start(out=prevh[:, 2:nb + 2], in_=prev_d[:])

        scores = cst.tile([BLOC, nb], F32, tag="scores")
        t1h = cst.tile([BLOC, nb + 4], F32, tag="t1h")
        out_sb = cst.tile([BLOC, nb], F32, tag="out")
        vconv = cst.tile([BLOC, nb], F32, tag="vconv")
        qTm = cst.tile([DK, BLOC * BLOC], F32R, tag="qTm")
        Zp = cst.tile([BLOC, nchunks], F32, tag="Zp")
        g_sb = cst.tile([BLOC, 1], F32, tag="g")
        c_sb = cst.tile([BLOC, 1], F32, tag="c1g")
        esd = cst.tile([BLOC, S], F32, tag="esd")
        zs = cst.tile([BLOC, 1], F32, tag="zs")
        rzs = cst.tile([BLOC, 1], F32, tag="rzs")
        sd_sb = cst.tile([BLOC, S], F32, tag="sd")
        z_sb = cst.tile([BLOC, 1], F32, tag="z")
        rz_sb = cst.tile([BLOC, 1], F32, tag="rz")
        a_sb = cst.tile([BLOC, 1], F32, tag="a")
        query = cst.tile([BLOC, DK], F32R, tag="query")
        inpT = cst.tile([DK, DK], F32R, tag="inpT")
        wT = cst.tile([DK, 4 * ODIM], F32R, tag="wT")

        # ---------------- stage A: args = input @ W.T + b ----------------
        # inputT: 4 transposes of input [32, 128c] -> psum [128, 4*32]
        it_ps = ps_a.tile([DK, DK], F32R, tag="psA")
        for t in range(4):
            nc.tensor.matmul(it_ps[:, t * BLOC:(t + 1) * BLOC],
                             inp_sb[:, t * DK:(t + 1) * DK], ident[0:BLOC, 0:BLOC],
                             is_transpose=True, start=(t == 0), stop=(t == 3))
        nc.vector.tensor_copy(inpT[:], it_ps[:])

        # WT chunks: [128dc, 134] per c
        for c in range(4):
            wt_ps = ps_a.tile([DK, ODIM], F32R, tag="psA")
            nc.tensor.matmul(wt_ps[:, 0:DK], w0_sb[:, c * DK:(c + 1) * DK],
                             ident[:], is_transpose=True, start=True, stop=False)
            nc.tensor.matmul(wt_ps[:, DK:ODIM], w1_sb[:, c * DK:(c + 1) * DK],
                             ident[0:ODIM - DK, 0:ODIM - DK],
                             is_transpose=True, start=False, stop=True)
            nc.vector.tensor_copy(wT[:, c * ODIM:(c + 1) * ODIM], wt_ps[:])

        args_ps = ps_a.tile([BLOC, ODIM], F32, tag="psA")
        for c in range(4):
            nc.tensor.matmul(args_ps[:],
                             inpT[:, c * BLOC:(c + 1) * BLOC],
                             wT[:, c * ODIM:(c + 1) * ODIM],
                             start=(c == 0), stop=False)
        nc.tensor.matmul(args_ps[:], ones_sb[:], bias_sb[:], start=False, stop=True)

        # gate / shift-dist / query
        nc.scalar.activation(g_sb[:], args_ps[:, DK:DK + 1], AF.Sigmoid)
        nc.scalar.activation(c_sb[:], g_sb[:], AF.Copy, bias=1.0, scale=-1.0)
        nc.scalar.activation(esd[:], args_ps[:, DK + 1:ODIM], AF.Exp,
                             accum_out=zs[:])
        nc.vector.reciprocal(rzs[:], zs[:])
        nc.scalar.activation(sd_sb[:], esd[:], AF.Copy, scale=rzs[:])
        nc.vector.tensor_copy(query[:], args_ps[:, 0:DK])

        qt_ps = ps_a.tile([DK, BLOC], F32R, tag="psA")
        nc.tensor.matmul(qt_ps[:], query[:], ident[0:BLOC, 0:BLOC],
                         is_transpose=True, start=True, stop=True)
        nc.vector.memset(qTm[:].bitcast(F32), 0.0)
        for b in range(BLOC):
            nc.vector.tensor_copy(qTm[:, b * BLOC + b:b * BLOC + b + 1],
                                  qt_ps[:, b:b + 1])

        # prev wraparound halo + v = conv5(sd, prev) on gpsimd (overlaps stream)
        nc.vector.tensor_copy(prevh[:, 0:2], prevh[:, nb:nb + 2])
        nc.vector.tensor_copy(prevh[:, nb + 2:nb + 4], prevh[:, 2:4])
        nc.scalar.mul(vconv[:], prevh[:, 0:nb], sd_sb[:, 0:1])
        for s in range(1, S):
            nc.vector.scalar_tensor_tensor(vconv[:], prevh[:, s:s + nb],
                                           sd_sb[:, s:s + 1], vconv[:],
                                           op0=OP.mult, op1=OP.add)

        # ---------------- stage B: stream memory ----------------
        for ci in range(nchunks):
            sc_ps = ps_sc.tile([BLOC, NCHUNK], F32, tag="psSC")
            for b in range(BLOC):
                nat = nat_pool.tile([DK, 4, DK], F32R, tag="nat")
                src = mem_d[b, ci * NCHUNK:(ci + 1) * NCHUNK, :]
                nc.sync.dma_start(out=nat[:],
                                  in_=src.rearrange("(p t) d -> p t d", t=4))
                tp_ps = ps_tp.tile([DK, NCHUNK], F32R, tag="psTP")
                for t in range(4):
                    nc.tensor.matmul(tp_ps[:, t * DK:(t + 1) * DK],
                                     nat[:, t, :], ident[:],
                                     is_transpose=True,
                                     start=(t == 0), stop=(t == 3))
                memT = mt_pool.tile([DK, NCHUNK], F32R, tag="memT")
                if act_memt_copies and (b % 2 == 1):
                    nc.scalar.copy(memT[:], tp_ps[:])
                else:
                    nc.vector.tensor_copy(memT[:], tp_ps[:])
                nc.tensor.matmul(sc_ps[:], qTm[:, b * BLOC:(b + 1) * BLOC],
                                 memT[:], start=(b == 0), stop=(b == BLOC - 1))
            # unpermute (col 128t+p <-> n 4p+t) into scores, then exp chunk
            dst = scores[:, ci * NCHUNK:(ci + 1) * NCHUNK]
            nc.scalar.copy(dst.rearrange("a (p t) -> a t p", t=4),
                           sc_ps[:].rearrange("a (t p) -> a t p", t=4))
            nc.scalar.activation(t1h[:, 2 + ci * NCHUNK:2 + (ci + 1) * NCHUNK],
                                 dst, AF.Exp, scale=SCALE,
                                 accum_out=Zp[:, ci:ci + 1])

        # ---------------- tail ----------------
        nc.vector.tensor_copy(t1h[:, 0:2], t1h[:, nb:nb + 2])
        nc.vector.tensor_copy(t1h[:, nb + 2:nb + 4], t1h[:, 2:4])
        nc.vector.reduce_sum(z_sb[:], Zp[:], axis=mybir.AxisListType.X)
        nc.vector.reciprocal(rz_sb[:], z_sb[:])
        nc.vector.tensor_mul(a_sb[:], g_sb[:], rz_sb[:])

        # u = conv5(sd, t1) into out_sb (ACT tap0 + DVE taps)
        nc.scalar.mul(out_sb[:], t1h[:, 0:nb], sd_sb[:, 0:1])
        for s in range(1, S):
            nc.vector.scalar_tensor_tensor(out_sb[:], t1h[:, s:s + nb],
                                           sd_sb[:, s:s + 1], out_sb[:],
                                           op0=OP.mult, op1=OP.add)
        # vconv *= (1-g)  (in-place ACT), then out = a*u + vconv
        nc.scalar.activation(vconv[:], vconv[:], AF.Copy, scale=c_sb[:])
        nc.vector.scalar_tensor_tensor(out_sb[:], out_sb[:], a_sb[:], vconv[:],
                                       op0=OP.mult, op1=OP.add)
        nc.sync.dma_start(out=out_d[:], in_=out_sb[:])

    if split:
        split_waits(nc)
    return nc


_PROGRAM = None


def _get_program():
    global _PROGRAM
    if _PROGRAM is None:
        _PROGRAM = build_program()
    return _PROGRAM


def _make_in_maps(inputs):
    inp = np.ascontiguousarray(np.asarray(inputs["input"], dtype=np.float32))
    mem = np.ascontiguousarray(np.asarray(inputs["memory"], dtype=np.float32))
    prev = np.ascontiguousarray(
        np.asarray(inputs["previous_addressing"], dtype=np.float32))
    W = np.ascontiguousarray(np.asarray(inputs["W"], dtype=np.float32))
    bvec = np.ascontiguousarray(np.asarray(inputs["b"], dtype=np.float32))
    shared = {
        "w0": W[0:DK], "w1": W[DK:ODIM], "bias": bvec.reshape(1, ODIM),
        "ones": np.ones((1, BLOC), np.float32),
        "ident": np.eye(DK, dtype=np.float32),
    }
    in_maps = []
    for core in range(NCORES):
        s = slice(core * BLOC, (core + 1) * BLOC)
        in_maps.append({"input": inp[s], "memory": mem[s], "prev": prev[s],
                        **shared})
    return in_maps


def _run(in_maps, **kwargs):
    return run_bass_kernel_spmd(_get_program(), in_maps,
                                core_ids=list(range(NCORES)), **kwargs)


_EXEC = {}


def _get_exec():
    """Cached jitted shard_map callable over the 8 cores (no donation, so it
    is re-invocable for timing; the kernel writes every element of out)."""
    if "fn" in _EXEC:
        return _EXEC
    import jax
    from jax.sharding import Mesh, PartitionSpec
    from jax.experimental.shard_map import shard_map
    from concourse import bass2jax, mybir as _mb

    bass2jax.install_neuronx_cc_hook()
    nc = _get_program()
    part_name = nc.partition_id_tensor.name if nc.partition_id_tensor else None
    in_names, out_names, out_avals = [], [], []
    for alloc in nc.m.functions[0].allocations:
        if not isinstance(alloc, _mb.MemoryLocationSet):
            continue
        name = alloc.memorylocations[0].name
        if alloc.kind == "ExternalInput":
            if name != part_name:
                in_names.append(name)
        elif alloc.kind == "ExternalOutput":
            out_names.append(name)
            out_avals.append(jax.core.ShapedArray(
                tuple(alloc.tensor_shape), _mb.dt.np(alloc.dtype)))
    n_params = len(in_names)
    all_names = in_names + out_names
    if part_name is not None:
        all_names = all_names + [part_name]

    def _body(*args):
        ops = list(args)
        if part_name is not None:
            ops.append(bass2jax.partition_id_tensor())
        outs = bass2jax._bass_exec_p.bind(
            *ops, out_avals=tuple(out_avals), in_names=tuple(all_names),
            out_names=tuple(out_names), lowering_input_output_aliases=(),
            sim_require_finite=True, sim_require_nnan=True, nc=nc)
        return tuple(outs)

    devices = jax.devices()[:NCORES]
    mesh = Mesh(np.asarray(devices), ("core",))
    nin = n_params + len(out_names)
    donate = tuple(range(n_params, n_params + len(out_names)))
    fn = jax.jit(shard_map(_body, mesh=mesh,
                           in_specs=(PartitionSpec("core"),) * nin,
                           out_specs=(PartitionSpec("core"),) * len(out_names),
                           check_rep=False),
                 donate_argnums=donate, keep_unused=True)
    _EXEC.update(fn=fn, in_names=in_names, out_names=out_names,
                 out_avals=out_avals, n_params=n_params, mesh=mesh)
    return _EXEC


def run_cached(in_maps):
    """Execute via the cached jitted callable; returns list of out dicts."""
    import jax
    ex = _get_exec()
    concat_in = [np.concatenate([np.asarray(m[name]) for m in in_maps], axis=0)
                 for name in ex["in_names"]]
    zeros = [np.zeros((NCORES * a.shape[0], *a.shape[1:]), a.dtype)
             for a in ex["out_avals"]]
    outs = ex["fn"](*concat_in, *zeros)
    jax.block_until_ready(outs)
    return [{name: np.asarray(outs[i]).reshape(NCORES, *ex["out_avals"][i].shape)[c]
             for i, name in enumerate(ex["out_names"])}
            for c in range(NCORES)]


def bench(in_maps, iters=8):
    """Time repeated executions with device-resident inputs; returns seconds list.
    Output (donated) buffers are pre-placed per iteration so the timed loop
    contains only dispatch + device execution."""
    import time as _t
    import jax
    from jax.sharding import NamedSharding, PartitionSpec
    ex = _get_exec()
    shard = NamedSharding(ex["mesh"], PartitionSpec("core"))
    concat_in = [np.concatenate([np.asarray(m[name]) for m in in_maps], axis=0)
                 for name in ex["in_names"]]
    args = [jax.device_put(a, shard) for a in concat_in]

    def mkzeros():
        return [jax.device_put(
                    np.zeros((NCORES * a.shape[0], *a.shape[1:]), a.dtype), shard)
                for a in ex["out_avals"]]

    jax.block_until_ready(ex["fn"](*args, *mkzeros()))  # warmup
    zsets = [mkzeros() for _ in range(iters)]
    jax.block_until_ready(zsets)
    times = []
    for i in range(iters):
        t0 = _t.perf_counter()
        jax.block_until_ready(ex["fn"](*args, *zsets[i]))
        times.append(_t.perf_counter() - t0)
    return times


def kernel(**inputs):
    res = _run(_make_in_maps(inputs))
    out = np.empty((B, NB), np.float32)
    for core in range(NCORES):
        out[core * BLOC:(core + 1) * BLOC] = res.results[core]["out"]
    return out
